# revision 13
# baseline (speedup 1.0000x reference)
"""CHRONOS GAT inference on 8 Trainium2 NeuronCores (Bass/Tile, SPMD).

Strategy (graph/data parallel, per sharding hint):
  * Host packs nodes into "tiles" of 16 dst slots with <=128 in-edges each
    (snake-deal by degree + repair), assigns tiles to cores balanced by edge
    count.  Nodes are relabeled so core c owns local ids [0, N_loc); the
    AllGather concatenation order defines the global permuted id.
  * Per GAT layer: a fused projection matmul produces per-node rows
    [hproj(256) | alpha_src(8) | alpha_dst(8)] in one pass (host premultiplies
    W @ blockdiag(a_src/a_dst)).  Rows [0:264] go to a DRAM shard, AllGather
    replicates them, and dma_gather pulls 320-float rows per edge.
  * Edge phase per super-tile (8 tiles x 128 edges): tiny mask matmuls
    broadcast alpha_dst / softmax denominators between edges and dst slots;
    softmax skips the max-subtraction (shift invariant; logits are O(1)).
    Aggregation is a mask matmul on the PE.
  * Layer 3 uses the factored form: aggregate alpha-weighted 256-dim g2
    messages per head (transposed), then per-head 256->256 matmul with W3;
    ELU is relu(x)+exp(min(x,0))-1 with the -1 folded into the classifier
    bias and the 1/8 head-mean folded into the classifier weight.
"""

import os
import sys

for _p in ("/opt/trn_rl_repo",):
    if os.path.isdir(_p) and _p not in sys.path:
        sys.path.insert(0, _p)

import numpy as np

import concourse.bass as bass
import concourse.mybir as mybir
import concourse.tile as tile
import concourse.bacc as bacc
from concourse.bass_utils import run_bass_kernel_spmd

F32 = mybir.dt.float32
I16 = mybir.dt.int16
AF = mybir.ActivationFunctionType
ALU = mybir.AluOpType

HEADS = 8
HID = 256
NEG = 0.2
D = 16          # dst slots per tile
KE = 128        # max edges per tile
ROW = 264       # shard row: 256 msg + 8 alpha_src
ROWP = 320      # padded row for dma_gather (bytes % 256 == 0)
P = 8           # cores


# ----------------------------------------------------------------------------
# host preprocessing
# ----------------------------------------------------------------------------

def _pack_nodes(deg, n):
    """tiles of <=16 nodes with <=128 total edges: snake deal + repair."""
    order = np.argsort(-deg, kind="stable")
    ntiles = -(-n // D)
    pad = ntiles * D - n
    order_p = np.concatenate([order, np.full(pad, -1, np.int64)])
    rows = order_p.reshape(D, ntiles).copy()
    rows[1::2] = rows[1::2, ::-1]
    tiles_arr = rows.T.copy()
    degp = np.concatenate([deg, [0]])
    sums = degp[tiles_arr].sum(1)
    for _ in range(100000):
        if sums.max() <= KE:
            break
        hot, cold = int(sums.argmax()), int(sums.argmin())
        hs = int(degp[tiles_arr[hot]].argmax())
        cs = int(degp[tiles_arr[cold]].argmin())
        a, b = tiles_arr[hot, hs], tiles_arr[cold, cs]
        if degp[a] <= degp[b]:
            raise RuntimeError("tile packing failed")
        tiles_arr[hot, hs], tiles_arr[cold, cs] = b, a
        sums = degp[tiles_arr].sum(1)
    assert sums.max() <= KE
    return tiles_arr, sums


def preprocess(edge_index, n):
    import heapq
    src = np.concatenate([edge_index[0].astype(np.int64), np.arange(n, dtype=np.int64)])
    dst = np.concatenate([edge_index[1].astype(np.int64), np.arange(n, dtype=np.int64)])
    deg = np.bincount(dst, minlength=n)
    assert deg.max() <= KE, f"in-degree {deg.max()} > {KE}"

    tiles_arr, sums = _pack_nodes(deg, n)
    ntiles = tiles_arr.shape[0]

    T_core = -(-ntiles // P)
    T_core = -(-T_core // 8) * 8
    core_tiles = [[] for _ in range(P)]
    load = [(0, c) for c in range(P)]
    heapq.heapify(load)
    for tid in sorted(range(ntiles), key=lambda t: -int(sums[t])):
        while True:
            ld, c = heapq.heappop(load)
            if len(core_tiles[c]) < T_core:
                core_tiles[c].append(tid)
                heapq.heappush(load, (ld + int(sums[tid]), c))
                break
    N_loc = T_core * D
    G = T_core // 8

    node_core = np.full(n, -1, np.int64)
    node_nloc = np.full(n, -1, np.int64)
    for c in range(P):
        for tl, tid in enumerate(core_tiles[c]):
            for s_, node in enumerate(tiles_arr[tid]):
                if node >= 0:
                    node_core[node] = c
                    node_nloc[node] = tl * D + s_
    assert (node_core >= 0).all()
    gid = node_core * N_loc + node_nloc

    idx_sb = np.zeros((P, 128, T_core * 8), np.int16)
    mask = np.zeros((P, 128, T_core * 16), np.float32)
    maskT = np.zeros((P, 128, G, 128), np.float32)
    ecore = node_core[dst]
    for c in range(P):
        sel = np.where(ecore == c)[0]
        d_loc = node_nloc[dst[sel]]
        t_of = d_loc // D
        slot = d_loc % D
        s_gid = gid[src[sel]]
        o = np.argsort(t_of, kind="stable")
        t_of, slot, s_gid = t_of[o], slot[o], s_gid[o]
        pos = np.zeros(len(sel), np.int64)
        cnt = np.zeros(T_core, np.int64)
        for i in range(len(sel)):
            t = t_of[i]
            pos[i] = cnt[t]
            cnt[t] += 1
        assert cnt.max() <= KE
        j = t_of * KE + pos
        idx_sb[c, j % 16, j // 16] = s_gid.astype(np.int16)
        mask[c, pos, t_of * 16 + slot] = 1.0
        maskT[c, (t_of % 8) * 16 + slot, t_of // 8, pos] = 1.0
    # dma_gather expects the 16-partition-wrapped index block replicated
    # across all 8 GPSIMD cores (partition groups of 16)
    idx_sb = np.tile(idx_sb[:, :16, :], (1, 8, 1))
    return dict(gid=gid, node_core=node_core, node_nloc=node_nloc,
                T_core=T_core, N_loc=N_loc, G=G,
                idx_sb=idx_sb, mask=mask,
                maskT=maskT.reshape(P, 128, G * 128), n=n)


def _expand_attn(a, C):
    out = np.zeros((HEADS * C, HEADS), np.float32)
    a = np.asarray(a, np.float32)
    for h in range(HEADS):
        out[h * C:(h + 1) * C, h] = a[h]
    return out


def prep_weights(inp, f_in):
    """host-side weight packing; layout + tiny transform matmuls."""
    w = {}
    w_in = np.zeros((256, 256), np.float32)
    w_in[:f_in] = np.asarray(inp["w_in"], np.float32)
    w["w_in"] = w_in
    for k in ("b_in", "b_t1", "b_t2", "b_g1", "b_g2", "b_g3", "b_c2"):
        w[k] = np.ascontiguousarray(np.asarray(inp[k], np.float32))
    w["w_t1"] = np.asarray(inp["w_t1"], np.float32)
    w["w_t2"] = np.asarray(inp["w_t2"], np.float32)
    for l in (1, 2):
        wg = np.asarray(inp[f"w_g{l}"], np.float32)
        w[f"wg{l}x"] = np.ascontiguousarray(np.concatenate(
            [wg, wg @ _expand_attn(inp[f"a_src{l}"], 32),
             wg @ _expand_attn(inp[f"a_dst{l}"], 32)], 1))
    wg3 = np.asarray(inp["w_g3"], np.float32)
    w["ws3"] = np.ascontiguousarray(
        np.concatenate([wg3 @ _expand_attn(inp["a_src3"], 256),
                        wg3 @ _expand_attn(inp["a_dst3"], 256)], 1))
    w["w_g3"] = wg3
    # fold 1/8 head-mean into classifier g-rows; fold elu's -1 into bias
    w_c1 = np.asarray(inp["w_c1"], np.float32).copy()
    b_c1 = np.asarray(inp["b_c1"], np.float32) - w_c1[:256].sum(0)
    w_c1[:256] /= 8.0
    w["w_c1"] = np.ascontiguousarray(w_c1)
    w["b_c1"] = np.ascontiguousarray(b_c1)
    w["w_c2"] = np.asarray(inp["w_c2"], np.float32)
    w["ident"] = np.eye(128, dtype=np.float32)
    # block-select: B[(t,d),(t',hd)] = (t == t')
    bsel = np.zeros((128, 64), np.float32)
    for t in range(8):
        bsel[t * 16:(t + 1) * 16, t * 8:(t + 1) * 8] = 1.0
    w["bsel"] = bsel
    return w


# ----------------------------------------------------------------------------
# device program
# ----------------------------------------------------------------------------

def build_program(T_core, n_cores=P, taps=False):
    NL = T_core * D
    G = T_core // 8
    nc = bacc.Bacc("TRN2", target_bir_lowering=False, debug=False,
                   num_devices=n_cores)

    def din(name, shape, dt=F32):
        return nc.dram_tensor(name, list(shape), dt, kind="ExternalInput").ap()

    io = dict(
        xT_d=din("xT", (256, NL)),
        idx_d=din("idx", (128, T_core * 8), I16),
        mask_d=din("maskv", (128, T_core * 16)),
        maskT_d=din("maskT", (128, G * 128)),
        w_in_d=din("w_in", (256, 256)), b_in_d=din("b_in", (256,)),
        w_t1_d=din("w_t1", (256, 256)), b_t1_d=din("b_t1", (256,)),
        w_t2_d=din("w_t2", (256, 256)), b_t2_d=din("b_t2", (256,)),
        wg1x_d=din("wg1x", (256, 272)), bg1_d=din("b_g1", (256,)),
        wg2x_d=din("wg2x", (256, 272)), bg2_d=din("b_g2", (256,)),
        ws3_d=din("ws3", (256, 16)),
        wg3_d=din("w_g3", (256, 2048)), bg3_d=din("b_g3", (2048,)),
        wc1_d=din("w_c1", (512, 256)), bc1_d=din("b_c1", (256,)),
        wc2_d=din("w_c2", (256, 2)), bc2_d=din("b_c2", (2,)),
        ident_d=din("ident", (128, 128)),
        bsel_d=din("bsel", (128, 64)),
        out_d=nc.dram_tensor("out", [128, G * 2], F32, kind="ExternalOutput").ap(),
    )
    if taps:
        for nm, shp in [("dbg_shard0", (NL, ROWP)), ("dbg_full0", (n_cores * NL, ROWP)),
                        ("dbg_mx", (128, 8 * ROWP)), ("dbg_ex8", (128, 64)),
                        ("dbg_alpha", (128, 64)), ("dbg_ade", (128, 64)),
                        ("dbg_g1T", (128, 2 * NL)), ("dbg_ad1", (128, G * 8))]:
            io[nm] = nc.dram_tensor(nm, list(shp), F32, kind="ExternalOutput").ap()
    io["taps"] = taps

    rg = [list(range(n_cores))]
    with tile.TileContext(nc) as tc:
        _body(nc, tc, io, T_core, NL, G, rg)
    nc.compile()
    return nc


def _body(nc, tc, io, T_core, NL, G, rg):
    ve = nc.vector
    sc = nc.scalar
    te = nc.tensor
    gp = nc.gpsimd
    sy = nc.sync

    pools = []

    def pool(**kw):
        cm = tc.tile_pool(**kw)
        p = cm.__enter__()
        pools.append(cm)
        return p

    wpool = pool(name="w", bufs=1)
    gpool = pool(name="graph", bufs=1)
    dram = pool(name="dram", bufs=1, space="DRAM")
    big = pool(name="big", bufs=2)
    tpool = pool(name="tres", bufs=1)
    grm_pool = pool(name="grm", bufs=1)
    adp = pool(name="adp", bufs=2)
    gtp = pool(name="gtp", bufs=1)
    ps_big = pool(name="ps_big", bufs=2, space="PSUM")
    ps_sm = pool(name="ps_sm", bufs=2, space="PSUM")
    ps_md = pool(name="ps_md", bufs=2, space="PSUM")
    sp = pool(name="sp", bufs=3)
    spe = pool(name="spe", bufs=2)
    rvp = pool(name="rvp", bufs=1)
    mxp = pool(name="mxp", bufs=2)
    mwp = pool(name="mwp", bufs=2)
    stg = pool(name="stg", bufs=2)

    def chunked(ap, m):
        return ap.rearrange("(c p) m -> p c m", p=128)

    def bvec(ap):
        return ap.rearrange("(c p) -> p c", p=128)

    def wtile(name, shape, view, dt=F32):
        t = wpool.tile(shape, dt, name=name)
        sy.dma_start(t[:], view)
        return t

    w_in_sb = wtile("w_in_sb", [128, 2, 256], chunked(io["w_in_d"], 256))
    w_t1_sb = wtile("w_t1_sb", [128, 2, 256], chunked(io["w_t1_d"], 256))
    w_t2_sb = wtile("w_t2_sb", [128, 2, 256], chunked(io["w_t2_d"], 256))
    wg1x_sb = wtile("wg1x_sb", [128, 2, 272], chunked(io["wg1x_d"], 272))
    wg2x_sb = wtile("wg2x_sb", [128, 2, 272], chunked(io["wg2x_d"], 272))
    ws3_sb = wtile("ws3_sb", [128, 2, 16], chunked(io["ws3_d"], 16))
    wg3_sb = wtile("wg3_sb", [128, 2, 2048], chunked(io["wg3_d"], 2048))
    wc1_sb = wtile("wc1_sb", [128, 4, 256], chunked(io["wc1_d"], 256))
    wc2_sb = wtile("wc2_sb", [128, 2, 2], chunked(io["wc2_d"], 2))
    ident = wtile("ident", [128, 128], io["ident_d"][:, :])

    b_in_sb = wtile("b_in_sb", [128, 2], bvec(io["b_in_d"]))
    b_t1_sb = wtile("b_t1_sb", [128, 2], bvec(io["b_t1_d"]))
    b_t2_sb = wtile("b_t2_sb", [128, 2], bvec(io["b_t2_d"]))
    bc1_sb = wtile("bc1_sb", [128, 2], bvec(io["bc1_d"]))
    bg3_sb = wtile("bg3_sb", [128, 16],
                   io["bg3_d"].rearrange("(hd half p) -> p (hd half)",
                                         hd=8, half=2, p=128))
    bg1_sb = wtile("bg1_sb", [128, 2], bvec(io["bg1_d"]))
    bg2_sb = wtile("bg2_sb", [128, 2], bvec(io["bg2_d"]))
    bsel_sb = wtile("bsel_sb", [128, 64], io["bsel_d"][:, :])

    def bcast_row(name, ap, m):
        row = wpool.tile([1, m], F32, name=name + "_row")
        sy.dma_start(row[:], ap[None, :])
        bc = wpool.tile([128, m], F32, name=name + "_bc")
        gp.partition_broadcast(bc[:], row[:])
        return bc

    bc2_bc = bcast_row("bc2", io["bc2_d"], 2)

    mask_sb = gpool.tile([128, T_core * 16], F32, name="mask_sb")
    sy.dma_start(mask_sb[:], io["mask_d"][:, :])
    maskT_sb = gpool.tile([128, G, 128], F32, name="maskT_sb")
    sy.dma_start(maskT_sb[:], io["maskT_d"].rearrange("p (g e) -> p g e", e=128))
    idx_sb = gpool.tile([128, T_core * 8], I16, name="idx_sb")
    sy.dma_start(idx_sb[:], io["idx_d"][:, :])

    shard = [dram.tile([NL, ROWP], F32, name=f"shard{l}", tag=f"shard{l}")
             for l in range(3)]
    full = [dram.tile([P * NL, ROWP], F32, name=f"full{l}", tag=f"full{l}",
                      addr_space="Shared") for l in range(3)]

    def nchunks():
        out, q = [], 0
        while q < NL:
            L = min(512, NL - q)
            out.append((q, L))
            q += L
        return out

    # dense transposed layer: out[:, co, q:q+L] from actT via weight wsb
    def dense_T(wsb, actT, outT, act, bias_sb):
        for co in range(2):
            for q, L in nchunks():
                ps = ps_big.tile([128, 512], F32, tag="psb")
                for ci in range(2):
                    te.matmul(ps[:, :L], wsb[:, ci, co * 128:(co + 1) * 128],
                              actT[:, ci, q:q + L], start=(ci == 0), stop=(ci == 1))
                bcol = bias_sb[:, co:co + 1]
                if act == "elu":
                    r = spe.tile([128, 512], F32, tag="elu_r")
                    u = spe.tile([128, 512], F32, tag="elu_u")
                    sc.activation(r[:, :L], ps[:, :L], AF.Relu, bias=bcol)
                    ve.tensor_scalar(out=u[:, :L], in0=ps[:, :L], scalar1=bcol,
                                     scalar2=0.0, op0=ALU.add, op1=ALU.min)
                    sc.activation(u[:, :L], u[:, :L], AF.Exp)
                    ve.scalar_tensor_tensor(out=outT[:, co, q:q + L], in0=r[:, :L],
                                            scalar=-1.0, in1=u[:, :L],
                                            op0=ALU.add, op1=ALU.add)
                elif act == "relu":
                    sc.activation(outT[:, co, q:q + L], ps[:, :L], AF.Relu,
                                  bias=bcol)
                else:
                    ve.tensor_scalar(out=outT[:, co, q:q + L], in0=ps[:, :L],
                                     scalar1=bcol, scalar2=None, op0=ALU.add)

    # ---------------- phase 0/1: h, t ----------------
    xT_sb = big.tile([128, 2, NL], F32, tag="big", name="xT_sb")
    sy.dma_start(xT_sb[:], io["xT_d"].rearrange("(c p) n -> p c n", p=128))
    hT_sb = big.tile([128, 2, NL], F32, tag="big", name="hT_sb")
    dense_T(w_in_sb, xT_sb, hT_sb, "elu", b_in_sb)
    t1T_sb = big.tile([128, 2, NL], F32, tag="big", name="t1T_sb")
    dense_T(w_t1_sb, hT_sb, t1T_sb, "relu", b_t1_sb)
    tT_sb = tpool.tile([128, 2, NL], F32, name="tT_sb")
    dense_T(w_t2_sb, t1T_sb, tT_sb, "bias", b_t2_sb)

    # ---------------- per-layer helpers ----------------
    def proj_phase(actT, wx_sb, layer, ad_sb):
        for g in range(G):
            ps = ps_md.tile([128, 272], F32, tag="md")
            for ci in range(2):
                te.matmul(ps[:], actT[:, ci, g * 128:(g + 1) * 128],
                          wx_sb[:, ci, :], start=(ci == 0), stop=(ci == 1))
            st = stg.tile([128, 264], F32, tag="stage")
            sc.activation(st[:], ps[:, :264], AF.Copy)
            ve.tensor_copy(ad_sb[:, g, :], ps[:, 264:272])
            sy.dma_start(shard[layer][g * 128:(g + 1) * 128, :264], st[:])

    def allgather(layer):
        gp.collective_compute("AllGather", ALU.bypass, replica_groups=rg,
                              ins=[shard[layer][:, :]], outs=[full[layer][:, :]])

    def edge_softmax(s, Mx, ad_sb, tap=False):
        # ad_sel[(t,d),(t',hd)] = ad[(t,d),hd] * (t==t'); one matmul broadcasts
        # per-dst values back to that tile's edges without cross-tile terms.
        ad_sel = sp.tile([128, 64], F32, tag="adsel")
        ad_b = ad_sb[:, s, :].unsqueeze(1).broadcast_to([128, 8, 8])
        ve.tensor_tensor(ad_sel[:].rearrange("p (t h) -> p t h", t=8),
                         ad_b, bsel_sb[:].rearrange("p (t h) -> p t h", t=8),
                         ALU.mult)
        ps_ade = ps_sm.tile([128, 64], F32, tag="sm")
        te.matmul(ps_ade[:], maskT_sb[:, s, :], ad_sel[:])
        ex8 = sp.tile([128, 64], F32, tag="ex8")
        ve.tensor_tensor(ex8[:].rearrange("p (t h) -> p t h", t=8),
                         Mx[:, :, 256:264],
                         ps_ade[:].rearrange("p (t h) -> p t h", t=8),
                         ALU.add)
        if tap:
            tmpade = sp.tile([128, 64], F32, tag="tapade")
            ve.tensor_copy(tmpade[:], ps_ade[:])
            sy.dma_start(io["dbg_ade"][:, :], tmpade[:])
        ve.scalar_tensor_tensor(out=ex8[:], in0=ex8[:], scalar=NEG,
                                in1=ex8[:], op0=ALU.mult, op1=ALU.max)
        sc.activation(ex8[:], ex8[:], AF.Exp)
        if tap:
            sy.dma_start(io["dbg_ex8"][:, :], ex8[:])
        ps_den = ps_sm.tile([128, 64], F32, tag="sm")
        te.matmul(ps_den[:], mask_sb[:, s * 128:(s + 1) * 128], ex8[:])
        den_sel = sp.tile([128, 64], F32, tag="den")
        ve.scalar_tensor_tensor(out=den_sel[:], in0=ps_den[:], scalar=0.0,
                                in1=bsel_sb[:], op0=ALU.add, op1=ALU.mult)
        ps_dene = ps_sm.tile([128, 64], F32, tag="sm")
        te.matmul(ps_dene[:], maskT_sb[:, s, :], den_sel[:])
        alpha8 = sp.tile([128, 64], F32, tag="alpha")
        ve.tensor_scalar(out=alpha8[:], in0=ps_dene[:], scalar1=1e-16,
                         scalar2=None, op0=ALU.add)
        ve.reciprocal(alpha8[:], alpha8[:])
        ve.tensor_mul(alpha8[:], alpha8[:], ex8[:])
        return alpha8

    def gather_Mx(layer, s):
        Mx = mxp.tile([128, 8, ROWP], F32, tag="mx")
        gp.dma_gather(Mx[:], full[layer][:, :], idx_sb[:, s * 64:(s + 1) * 64],
                      num_idxs=1024, num_idxs_reg=1024,
                      elem_size=ROWP, elem_step=ROWP)
        return Mx

    def edge_phase12(layer, ad_sb, gT_out, bg_sb):
        for s in range(G):
            Mx = gather_Mx(layer, s)
            alpha8 = edge_softmax(s, Mx, ad_sb)
            msgw = mwp.tile([128, 2048], F32, tag="mw")
            a_b = alpha8[:].rearrange("p (t h) -> p t h", t=8)
            a_b = a_b.unsqueeze(3).broadcast_to([128, 8, 8, 32])
            m_b = Mx[:, :, 0:256].rearrange("p t (h c) -> p t h c", h=8)
            ve.tensor_tensor(msgw[:].rearrange("p (t h c) -> p t h c", t=8, h=8),
                             m_b, a_b, ALU.mult)
            # transposed aggregation: ZT[:, c*128 + t*16+d]
            ps_z = ps_md.tile([128, 256], F32, tag="md")
            for t in range(8):
                for c in range(2):
                    te.matmul(ps_z[:, c * 128 + t * 16:c * 128 + (t + 1) * 16],
                              msgw[:, t * 256 + c * 128:t * 256 + (c + 1) * 128],
                              mask_sb[:, s * 128 + t * 16:s * 128 + (t + 1) * 16])
            for c in range(2):
                bcol = bg_sb[:, c:c + 1]
                zc = ps_z[:, c * 128:(c + 1) * 128]
                r = sp.tile([128, 128], F32, tag="zr")
                sc.activation(r[:], zc, AF.Relu, bias=bcol)
                u = sp.tile([128, 128], F32, tag="zu")
                ve.tensor_scalar(out=u[:], in0=zc, scalar1=bcol,
                                 scalar2=0.0, op0=ALU.add, op1=ALU.min)
                sc.activation(u[:], u[:], AF.Exp)
                ve.scalar_tensor_tensor(out=gT_out[:, c, s * 128:(s + 1) * 128],
                                        in0=r[:], scalar=-1.0, in1=u[:],
                                        op0=ALU.add, op1=ALU.add)

    def transpose_rm(inT, out_rm):
        """transposed [128, 2, NL] -> row-major [128, G, 256]"""
        for g in range(G):
            for c in range(2):
                ps_t = ps_md.tile([128, 128], F32, tag="md")
                te.transpose(ps_t[:], inT[:, c, g * 128:(g + 1) * 128], ident[:])
                ve.tensor_copy(out_rm[:, g, c * 128:(c + 1) * 128], ps_t[:])

    # ---------------- layer 1 ----------------
    ad1 = adp.tile([128, G, 8], F32, tag="ad", name="ad1")
    proj_phase(hT_sb, wg1x_sb, 0, ad1)
    allgather(0)
    g1T = big.tile([128, 2, NL], F32, tag="big", name="g1T")
    if io.get("taps"):
        sy.dma_start(io["dbg_shard0"][:, :], shard[0][:, :])
        sy.dma_start(io["dbg_full0"][:, :], full[0][:, :])
        sy.dma_start(io["dbg_ad1"][:, :], ad1[:].rearrange("p g a -> p (g a)"))
        Mx0 = gather_Mx(0, 0)
        sy.dma_start(io["dbg_mx"][:, :], Mx0[:].rearrange("p t r -> p (t r)"))
        al0 = edge_softmax(0, Mx0, ad1, tap=True)
        sy.dma_start(io["dbg_alpha"][:, :], al0[:])
    edge_phase12(0, ad1, g1T, bg1_sb)
    if io.get("taps"):
        sy.dma_start(io["dbg_g1T"][:, :], g1T[:].rearrange("p c n -> p (c n)"))

    # ---------------- layer 2 ----------------
    ad2 = adp.tile([128, G, 8], F32, tag="ad", name="ad2")
    proj_phase(g1T, wg2x_sb, 1, ad2)
    allgather(1)
    g2T = big.tile([128, 2, NL], F32, tag="big", name="g2T")
    edge_phase12(1, ad2, g2T, bg2_sb)

    # ---------------- layer 3 prep ----------------
    g2_rm = big.tile([128, G, 256], F32, tag="big", name="g2_rm")
    transpose_rm(g2T, g2_rm)
    ad3 = adp.tile([128, G, 8], F32, tag="ad", name="ad3")
    for g in range(G):
        ps = ps_sm.tile([128, 16], F32, tag="sm")
        for ci in range(2):
            te.matmul(ps[:], g2T[:, ci, g * 128:(g + 1) * 128],
                      ws3_sb[:, ci, :], start=(ci == 0), stop=(ci == 1))
        st = stg.tile([128, 264], F32, tag="stage")
        sc.activation(st[:, :256], g2_rm[:, g, :], AF.Copy)
        ve.tensor_copy(st[:, 256:264], ps[:, 0:8])
        ve.tensor_copy(ad3[:, g, :], ps[:, 8:16])
        sy.dma_start(shard[2][g * 128:(g + 1) * 128, :264], st[:])
    allgather(2)

    # ---------------- layer 3 edge phase ----------------
    gT_sb = gtp.tile([128, 2, NL], F32, name="gT_sb")
    for s in range(G):
        Mx = gather_Mx(2, s)
        alpha8 = edge_softmax(s, Mx, ad3)
        # transposed aggregation: ZT[:, c*1024 + t*128 + (d*8+hd)]
        ZT = mwp.tile([128, 2048], F32, tag="mw")
        for t in range(8):
            Ap = sp.tile([128, 128], F32, tag="Ap")
            m_b = mask_sb[:, s * 128 + t * 16:s * 128 + (t + 1) * 16]
            m_b = m_b.unsqueeze(2).broadcast_to([128, 16, 8])
            a_b = alpha8[:, t * 8:(t + 1) * 8]
            a_b = a_b.unsqueeze(1).broadcast_to([128, 16, 8])
            ve.tensor_tensor(Ap[:].rearrange("p (d h) -> p d h", d=16),
                             m_b, a_b, ALU.mult)
            for c in range(2):
                ps_zt = ps_md.tile([128, 128], F32, tag="md")
                te.matmul(ps_zt[:], Mx[:, t, c * 128:(c + 1) * 128], Ap[:])
                ve.tensor_copy(ZT[:, c * 1024 + t * 128:c * 1024 + (t + 1) * 128],
                               ps_zt[:])
        # W3 stage, transposed; accumulate 8 heads' relu/exp parts in slots
        for half in range(2):
            RV = rvp.tile([128, 128, 16], F32, tag="rv")
            for hd in range(8):
                ps_o = ps_md.tile([128, 128], F32, tag="md")
                for c in range(2):
                    # rhs: ZT columns (t, d) at fixed (c, hd): c*1024 + t*128 + d*8 + hd
                    zt = ZT[:].rearrange("p (c t d h) -> p c t d h", c=2, t=8, d=16)
                    zt = zt[:, c, :, :, hd]       # [128, t, d]
                    te.matmul(ps_o[:],
                              wg3_sb[:, c, hd * 256 + half * 128:
                                     hd * 256 + (half + 1) * 128],
                              zt.rearrange("p t d -> p (t d)"),
                              start=(c == 0), stop=(c == 1))
                bcol = bg3_sb[:, hd * 2 + half:hd * 2 + half + 1]
                sc.activation(RV[:, :, hd * 2], ps_o[:], AF.Relu, bias=bcol)
                u3 = sp.tile([128, 128], F32, tag="u3")
                ve.tensor_scalar(out=u3[:], in0=ps_o[:], scalar1=bcol,
                                 scalar2=0.0, op0=ALU.add, op1=ALU.min)
                sc.activation(RV[:, :, hd * 2 + 1], u3[:], AF.Exp)
            # gT = sum over 16 slots  (= sum_hd relu+exp; -8 & /8 folded in w_c1/b_c1)
            ve.tensor_reduce(gT_sb[:, half, s * 128:(s + 1) * 128],
                             RV[:], mybir.AxisListType.X, ALU.add)

    # ---------------- classifier ----------------
    c1T = big.tile([128, 2, NL], F32, tag="big", name="c1T")
    for co in range(2):
        for q, L in nchunks():
            ps = ps_big.tile([128, 512], F32, tag="psb")
            srcs = [(gT_sb, 0), (gT_sb, 1), (tT_sb, 0), (tT_sb, 1)]
            for ci, (srcT, cc) in enumerate(srcs):
                te.matmul(ps[:, :L], wc1_sb[:, ci, co * 128:(co + 1) * 128],
                          srcT[:, cc, q:q + L], start=(ci == 0), stop=(ci == 3))
            sc.activation(c1T[:, co, q:q + L], ps[:, :L], AF.Relu,
                          bias=bc1_sb[:, co:co + 1])

    lg_sb = adp.tile([128, G, 2], F32, tag="lg", name="lg_sb")
    for g in range(G):
        ps = ps_sm.tile([128, 2], F32, tag="sm")
        for ci in range(2):
            te.matmul(ps[:], c1T[:, ci, g * 128:(g + 1) * 128],
                      wc2_sb[:, ci, :], start=(ci == 0), stop=(ci == 1))
        ve.tensor_tensor(lg_sb[:, g, :], ps[:], bc2_bc[:, 0:2], ALU.add)
    sy.dma_start(io["out_d"][:, :], lg_sb[:].rearrange("p g c -> p (g c)"))

    for p_ in reversed(pools):
        p_.__exit__(None, None, None)


# ----------------------------------------------------------------------------
# entry point
# ----------------------------------------------------------------------------

def make_in_maps(inputs, pre, w):
    """per-core input dicts."""
    n = pre["n"]
    NL = pre["N_loc"]
    x = np.asarray(inputs["x"], np.float32)
    f_in = x.shape[1]
    in_maps = []
    for c in range(P):
        xT = np.zeros((256, NL), np.float32)
        owned = np.where(pre["node_core"] == c)[0]
        xT[:f_in, pre["node_nloc"][owned]] = x[owned].T
        m = dict(
            xT=xT, idx=pre["idx_sb"][c], maskv=pre["mask"][c],
            maskT=pre["maskT"][c],
            w_in=w["w_in"], b_in=w["b_in"], w_t1=w["w_t1"], b_t1=w["b_t1"],
            w_t2=w["w_t2"], b_t2=w["b_t2"], wg1x=w["wg1x"], b_g1=w["b_g1"],
            wg2x=w["wg2x"], b_g2=w["b_g2"], ws3=w["ws3"], w_g3=w["w_g3"],
            b_g3=w["b_g3"], w_c1=w["w_c1"], b_c1=w["b_c1"], w_c2=w["w_c2"],
            b_c2=w["b_c2"], ident=w["ident"], bsel=w["bsel"],
        )
        in_maps.append({k: np.ascontiguousarray(v) for k, v in m.items()})
    return in_maps


def unshard_output(results, pre):
    n = pre["n"]
    G = pre["G"]
    out = np.zeros((n, 2), np.float32)
    for c in range(P):
        lg = results[c]["out"].reshape(128, G, 2)
        owned = np.where(pre["node_core"] == c)[0]
        nloc = pre["node_nloc"][owned]
        out[owned] = lg[nloc % 128, nloc // 128, :]
    return out


_PROGRAM_CACHE = {}
LAST_RESULT = None
LAST_TIMING = {}


def kernel(**inputs):
    import time as _time
    t0 = _time.time()
    x = np.asarray(inputs["x"])
    edge_index = np.asarray(inputs["edge_index"])
    n, f_in = x.shape
    pre = preprocess(edge_index, n)
    w = prep_weights(inputs, f_in)
    t1 = _time.time()
    key = pre["T_core"]
    if key not in _PROGRAM_CACHE:
        _PROGRAM_CACHE[key] = build_program(pre["T_core"])
    nc = _PROGRAM_CACHE[key]
    t2 = _time.time()
    in_maps = make_in_maps(inputs, pre, w)
    t3 = _time.time()
    res = run_bass_kernel_spmd(nc, in_maps, core_ids=list(range(P)))
    t4 = _time.time()
    global LAST_RESULT
    LAST_RESULT = res
    out = unshard_output(res.results, pre)
    LAST_TIMING.update(preprocess_s=t1 - t0, build_s=t2 - t1,
                       inmaps_s=t3 - t2, execute_s=t4 - t3,
                       total_s=_time.time() - t0)
    return out


if __name__ == "__main__":
    import reference
    inputs = {k: np.asarray(v) for k, v in reference.setup_inputs().items()}
    expected = np.asarray(reference.reference(**inputs))
    actual = kernel(**inputs)
    err = np.abs(actual - expected)
    print("abs max err:", err.max(), "rel:", err.max() / np.abs(expected).max())
